# revision 87
# baseline (speedup 1.0000x reference)
"""AggregationDiscriminationLoss kernel for 8 TRN2 NeuronCores.

Data-parallel over batch N=8 (one sample per core). Per core, pixels live in
[128, 3200] bf16 planes (P = 640*640):

- Segment sums (G[m,c], cnt_k): per-column PE matmuls — stationary =
  [4 sim channels + ones] at column q, moving = 15 one-hot columns of the
  kern ids (generated by DVE 4x eq-passes), PSUM-accumulated over all q.
- Gather G[text[p]]: 32-row slabs x 4 m-passes. Text ids are replicated 4x
  across partition sub-slots via a DRAM bounce, compared against per-
  partition m values (one DVE 4x pass per m-pass), then multiplied by a
  block-diagonal G table on the PE, accumulating all 16 ids in PSUM; the
  result is scattered back to pixel layout by DMA.
- Per-pixel loss chain on ACT (sqrt/relu/square/ln) + DVE diffs.
- Masked l-sums + cnt_t: per-column PE matmuls with a [l | ones] stationary,
  so one PSUM accumulation yields both, already reduced over all partitions.
- Pairwise-distance (dis) chain runs on partition 0 only, overlapping the
  gather; all final combines are partition-0 tinies.
Work is spread across DVE/ACT/Pool/PE; outputs (agg_i, dis_i) per core.
"""

import numpy as np

import concourse.bacc as bacc
import concourse.mybir as mybir
import concourse.tile as tile
from concourse import bass_utils

F32 = mybir.dt.float32
BF16 = mybir.dt.bfloat16
I32 = mybir.dt.int32
A = mybir.AluOpType
ACTF = mybir.ActivationFunctionType

M = 16
DELTA_AGG = 0.5
DELTA_DIS = 3.0
H = W = 640
P = H * W            # 409600
PARTS = 128
FREE = P // PARTS    # 3200
NM = M - 1           # ids 1..15 (id 0 never contributes to the losses)
GSPLIT = 1           # column splits of the gather pipeline

# engine-routing knobs, tuned against the TimelineSim cost model.
CFG = {
    "oh_pool_every": 4,   # every k-th kern one-hot pass on GpSimd
    "oht_pool_every": 99,  # every k-th text one-hot pass on GpSimd
}


def build_kernel_body(tc, out_ap, sim_ap, tgt_ap, ne_ap, miotas_ap, bmask_ap):
    nc = tc.nc

    simr = sim_ap.rearrange("c (p f) -> c p f", p=PARTS)   # (4, 128, 3200)
    tgtr = tgt_ap.rearrange("c (p f) -> c p f", p=PARTS)   # (2, 128, 3200)

    with tc.tile_pool(name="big", bufs=1) as big, \
         tc.tile_pool(name="stage", bufs=5) as stagep, \
         tc.tile_pool(name="ohp", bufs=2) as ohpool, \
         tc.tile_pool(name="kl", bufs=1) as klp, \
         tc.tile_pool(name="reps", bufs=2) as repp, \
         tc.tile_pool(name="stg", bufs=2) as stgp, \
         tc.tile_pool(name="ohs", bufs=2) as ohsp, \
         tc.tile_pool(name="gps", bufs=4, space="PSUM") as gpsp, \
         tc.tile_pool(name="dram", bufs=1, space="DRAM") as dramp, \
         tc.tile_pool(name="psum", bufs=1, space="PSUM") as psp, \
         tc.tile_pool(name="small", bufs=1) as small:

        NCH = 5
        CF = FREE // NCH

        # ---------- chunked loads + casts (kern first so the PE pipeline
        # can start as soon as chunk 0 of scv/oh is ready) ----------
        K16 = klp.tile([PARTS, FREE], BF16, tag="kl", name="K16")
        T16 = big.tile([PARTS, FREE], BF16, tag="T16")
        scv = big.tile([PARTS, 5, FREE], BF16, tag="scv")
        sc = [scv[:, c, :] for c in range(4)]
        nc.gpsimd.memset(scv[:, 4, :], 1.0)
        mio = small.tile([PARTS, 4], F32, tag="mio")
        nc.sync.dma_start(mio[:], miotas_ap)
        bmf = small.tile([PARTS, PARTS], F32, tag="bmf")
        nc.sync.dma_start(bmf[:], bmask_ap)
        bm16 = small.tile([PARTS, PARTS], BF16, tag="bm16")
        nc.vector.tensor_copy(bm16[:], bmf[:])

        tdr = dramp.tile([PARTS, FREE], BF16, tag="tdr")
        for ch in range(NCH):
            q0 = ch * CF
            k_stage = stagep.tile([PARTS, CF], I32, tag="stage", name="kst")
            nc.sync.dma_start(k_stage[:], tgtr[1][:, q0:q0 + CF])
            nc.gpsimd.tensor_copy(K16[:, q0:q0 + CF], k_stage[:])
            for c in range(4):
                s_stage = stagep.tile([PARTS, CF], F32, tag="stage",
                                      name="sst")
                nc.sync.dma_start(s_stage[:], simr[c][:, q0:q0 + CF])
                nc.scalar.copy(sc[c][:, q0:q0 + CF], s_stage[:])
        for ch in range(NCH):
            q0 = ch * CF
            t_stage = stagep.tile([PARTS, CF], I32, tag="stage", name="tst")
            nc.sync.dma_start(t_stage[:], tgtr[0][:, q0:q0 + CF])
            nc.scalar.copy(T16[:, q0:q0 + CF], t_stage[:])

        nc.sync.dma_start(tdr[:], T16[:])
        tdrv = tdr[:].rearrange("(t g) q -> t g q", g=32)

        # bias constants for ACT ops
        bm_agg = small.tile([PARTS, 1], F32, tag="bm_agg")
        nc.gpsimd.memset(bm_agg[:], -DELTA_AGG)
        bm_dis = small.tile([PARTS, 1], F32, tag="bm_dis")
        nc.gpsimd.memset(bm_dis[:], DELTA_DIS)

        # engine load balancer (estimated busy ns per engine); each option is
        # a dict engine -> added busy ns, chosen to minimize the peak load.
        load = {"dve": 0.0, "act": 0.0, "pool": 0.0}

        def pick(options):
            def peak(opt):
                return max(load[e] + opt.get(e, 0.0) for e in load)
            name = min(options, key=lambda k: peak(options[k]))
            for e, v in options[name].items():
                load[e] += v
            return name

        # ---------- phase A ----------
        # cnt_k / cnt_t via DVE 4x eq-passes (accum riding); G sums on the PE
        # as per-column matmuls: lhsT = scv[:, :, q] (4 sim channels), rhs =
        # one-hot of kern ids (15 m-columns), accumulated in PSUM over all q.
        ps = psp.tile([5, NM], F32, tag="ps")
        qg = 0
        for ch in range(NCH):
            q0 = ch * CF
            oh = ohpool.tile([PARTS, NM, CF], BF16, tag="oh", name="oh")
            for m in range(1, M):
                eng = "pool" if m % CFG["oh_pool_every"] == 0 else "dve"
                if eng == "pool":
                    nc.gpsimd.tensor_scalar(
                        oh[:, m - 1, :], K16[:, q0:q0 + CF], float(m), None,
                        A.is_equal)
                else:
                    nc.vector.tensor_scalar(
                        oh[:, m - 1, :], K16[:, q0:q0 + CF], float(m), None,
                        A.is_equal)
            for q in range(CF):
                nc.tensor.matmul(
                    ps[:], scv[:, :, q0 + q:q0 + q + 1], oh[:, :, q:q + 1],
                    start=(qg == 0), stop=(qg == FREE - 1))
                qg += 1

        # raw G sums + cnt_k (ones row): PSUM -> SBUF -> one row -> broadcast
        pss = small.tile([5, NM], F32, tag="pss")
        nc.vector.tensor_copy(pss[:], ps[:])
        g1row = small.tile([1, 5 * NM], F32, tag="g1row")
        nc.sync.dma_start(g1row[:], pss[:])
        # gather-table path first (row 0 only, no broadcast dependency)
        mk0 = small.tile([1, NM], F32, tag="mk0")
        nc.vector.tensor_scalar(mk0[:], g1row[0:1, 4 * NM:5 * NM], 1.0,
                                None, A.max)
        rk0 = small.tile([1, NM], F32, tag="rk0")
        nc.vector.reciprocal(rk0[:], mk0[:])
        g0 = small.tile([1, 4 * NM], F32, tag="g0")
        nc.vector.tensor_tensor(
            g0[:].rearrange("p (c m) -> p c m", c=4),
            g1row[0:1, 0:4 * NM].rearrange("p (c m) -> p c m", c=4),
            rk0[:].unsqueeze(1).broadcast_to([1, 4, NM]),
            A.mult)


        # ---------- phase C: gather G[text] on the PE ----------
        # 32-row slabs x 4 m-passes: text ids are replicated 4x across
        # partition sub-slots (DRAM bounce), compared against per-partition
        # m values (one 4x DVE pass per m-pass), and multiplied by a
        # block-diagonal G table on the tensor engine, accumulating all 16
        # ids in PSUM. Result is scattered back to pixel layout by DMA.
        # G table (m-major, m=0 zero) -> DRAM bounce
        gsb = small.tile([1, 64], BF16, tag="gsb")
        nc.vector.memset(gsb[:], 0.0)
        nc.vector.tensor_copy(
            gsb[0:1, 4:64].rearrange("p (m c) -> p m c", c=4),
            g0[:].rearrange("p (c m) -> p m c", c=4))
        gd = dramp.tile([1, 64], BF16, tag="gd")
        nc.sync.dma_start(gd[:], gsb[:])
        gdv = gd[:].rearrange("one (m c) -> one m c", c=4)
        gblks = []
        for mp in range(4):
            grow = small.tile([PARTS, 4], BF16, tag=f"grow{mp}",
                              name=f"grow{mp}")
            nc.sync.dma_start(
                grow[:],
                gdv[:, 4 * mp:4 * mp + 4, :].broadcast_to([32, 4, 4]))
            gb = small.tile([PARTS, PARTS], BF16, tag=f"gblk{mp}",
                            name=f"gblk{mp}")
            nc.vector.tensor_tensor(
                gb[:].rearrange("p (gp c) -> p gp c", c=4),
                grow[:].unsqueeze(1).broadcast_to([PARTS, 32, 4]),
                bm16[:].rearrange("p (gp c) -> p gp c", c=4),
                A.mult)
            gblks.append(gb)

        QG = 400
        GH = FREE // GSPLIT
        gtv = big.tile([PARTS, 4, FREE], BF16, tag="gtv")
        for gh in range(GSPLIT):
            h0 = gh * GH
            for t in range(4):
                rep = repp.tile([PARTS, GH], BF16, tag="rep", name="rep")
                nc.sync.dma_start(
                    rep[:], tdrv[t, :, h0:h0 + GH].unsqueeze(1)
                    .broadcast_to([32, 4, GH]))
                ohs = []
                for mp in range(4):
                    oh4 = ohsp.tile([PARTS, GH], BF16, tag=f"oh4_{mp}",
                                    name=f"oh4_{mp}")
                    nc.vector.tensor_scalar(
                        oh4[:], rep[:], mio[:, mp:mp + 1], None, A.is_equal)
                    load["dve"] += 500
                    ohs.append(oh4)
                stg = stgp.tile([PARTS, GH], BF16, tag="stg", name="stg")
                for qc in range(GH // QG):
                    q0 = qc * QG
                    psg = gpsp.tile([PARTS, QG], F32, tag="psg", name="psg")
                    for mp in range(4):
                        nc.tensor.matmul(
                            psg[:], gblks[mp][:], ohs[mp][:, q0:q0 + QG],
                            start=(mp == 0), stop=(mp == 3))
                    eng = pick({"dve": {"dve": 650}, "act": {"act": 450}})
                    if eng == "act":
                        nc.scalar.copy(stg[:, q0:q0 + QG], psg[:])
                    else:
                        nc.vector.tensor_copy(stg[:, q0:q0 + QG], psg[:])
                for j in range(4):
                    nc.sync.dma_start(
                        gtv[32 * t + 8 * j:32 * t + 8 * (j + 1), :,
                            h0:h0 + GH],
                        stg[32 * j:32 * (j + 1), :])
        gt = [gtv[:, c, :] for c in range(4)]

        # ---------- dis heavy part (needs only G; overlaps gather/tail) ----
        # forward-equivalent without the where(pair, sq, 1) guard: invalid
        # pairs produce finite values that are masked after the fact.
        NP = NM * NM
        ne_s = small.tile([1, NP], F32, tag="ne_s")
        nc.sync.dma_start(ne_s[:], ne_ap)
        dif = stgp.tile([1, NP * 4], F32, tag="stg", name="dif")
        g_m = g0[:].rearrange("p (c m) -> p m c", c=4).unsqueeze(2)
        g_mp = g0[:].rearrange("p (c m) -> p m c", c=4).unsqueeze(1)
        nc.vector.tensor_tensor(
            dif[:].rearrange("p (m n c) -> p m n c", m=NM, n=NM),
            g_m.broadcast_to([1, NM, NM, 4]),
            g_mp.broadcast_to([1, NM, NM, 4]),
            A.subtract)
        nc.vector.tensor_tensor(dif[:], dif[:], dif[:], A.mult)
        lp = small.tile([1, NP], F32, tag="lp")
        nc.vector.tensor_reduce(
            lp[:], dif[:].rearrange("p (n c) -> p n c", c=4),
            mybir.AxisListType.X, A.add)
        nc.scalar.activation(lp[:], lp[:], ACTF.Sqrt)
        nc.scalar.activation(lp[:], lp[:], ACTF.Relu, bias=bm_dis[0:1, :],
                             scale=-1.0)
        nc.vector.tensor_tensor(lp[:], lp[:], lp[:], A.mult)
        nc.scalar.activation(lp[:], lp[:], ACTF.Ln, bias=1.0)

        # ---------- tail loop 1: diff/sq/d2/sqrt/u/u2 per chunk ----------
        lpl = scv[:, 3, :]   # sim plane 3 is dead after its diff
        for ch in range(NCH):
            q0 = ch * CF
            s_ = slice(q0, q0 + CF)
            for c in range(4):
                nc.vector.tensor_tensor(gt[c][:, s_], sc[c][:, s_],
                                        gt[c][:, s_], A.subtract)
                load["dve"] += 330
                if c == 0:
                    nc.scalar.square(gt[c][:, s_], gt[c][:, s_])
                elif c == 99:
                    nc.gpsimd.tensor_tensor(gt[c][:, s_], gt[c][:, s_],
                                            gt[c][:, s_], A.mult)
                else:
                    nc.vector.tensor_tensor(gt[c][:, s_], gt[c][:, s_],
                                            gt[c][:, s_], A.mult)
            nc.vector.tensor_tensor(gt[0][:, s_], gt[0][:, s_],
                                    gt[1][:, s_], A.add)
            nc.vector.tensor_tensor(gt[2][:, s_], gt[2][:, s_],
                                    gt[3][:, s_], A.add)
            nc.vector.tensor_tensor(gt[0][:, s_], gt[0][:, s_],
                                    gt[2][:, s_], A.add)  # d2
            load["dve"] += 3 * 330
            nc.scalar.activation(gt[1][:, s_], gt[0][:, s_], ACTF.Sqrt)
            nc.scalar.activation(gt[2][:, s_], gt[1][:, s_], ACTF.Relu,
                                 bias=bm_agg[:])                     # u
            nc.scalar.activation(gt[3][:, s_], gt[2][:, s_], ACTF.Square)
            load["act"] += 3 * 700

        # ---------- tail loop 2: ln + one-hots + PE masked-l sums ----------
        # stationary carries [l | ones] so the same matmuls also produce
        # cnt_t globally (no partition reduction needed).
        oht_tiles = []
        for ch in range(NCH):
            q0 = ch * CF
            s_ = slice(q0, q0 + CF)
            oht = ohpool.tile([PARTS, NM, CF], BF16, tag="oh", name="oht")
            for m in range(1, M):
                if m % CFG["oht_pool_every"] == 0:
                    nc.gpsimd.tensor_scalar(
                        oht[:, m - 1, :], T16[:, s_], float(m), None,
                        A.is_equal)
                else:
                    nc.vector.tensor_scalar(
                        oht[:, m - 1, :], T16[:, s_], float(m), None,
                        A.is_equal)
            oht_tiles.append(oht)
            nc.scalar.activation(lpl[:, s_], gt[3][:, s_], ACTF.Ln,
                                 bias=1.0)
            load["act"] += 700
        ps2 = psp.tile([2, NM], F32, tag="ps2")
        qg = 0
        for ch in range(NCH):
            q0 = ch * CF
            oht = oht_tiles[ch]
            for q in range(CF):
                nc.tensor.matmul(
                    ps2[:], scv[:, 3:5, q0 + q:q0 + q + 1],
                    oht[:, :, q:q + 1],
                    start=(qg == 0), stop=(qg == FREE - 1))
                qg += 1
        lred = small.tile([2, NM], F32, tag="lred")
        nc.vector.tensor_copy(lred[:], ps2[:])
        l1row = small.tile([1, 2 * NM], F32, tag="l1row")
        nc.sync.dma_start(l1row[:], lred[:])

        # ---------- final combines, all on partition 0 ----------
        ck0 = g1row[0:1, 4 * NM:5 * NM]
        ls0 = l1row[0:1, 0:NM]
        ct0 = l1row[0:1, NM:2 * NM]
        mt0 = small.tile([1, NM], F32, tag="mt0")
        nc.vector.tensor_scalar(mt0[:], ct0, 1.0, None, A.max)
        rt0 = small.tile([1, NM], F32, tag="rt0")
        nc.vector.reciprocal(rt0[:], mt0[:])
        vk0 = small.tile([1, NM], F32, tag="vk0")
        nc.vector.tensor_scalar(vk0[:], ck0, 0.0, None, A.is_gt)
        v0 = small.tile([1, NM], F32, tag="v0")
        nc.vector.tensor_scalar(v0[:], ct0, 0.0, None, A.is_gt)
        nc.vector.tensor_tensor(v0[:], v0[:], vk0[:], A.mult)
        nv0 = small.tile([1, 1], F32, tag="nv0")
        nc.vector.tensor_reduce(nv0[:], v0[:], mybir.AxisListType.X, A.add)

        # agg = sum(valid * l_sum / max(cnt_t,1)) / max(nv, 1)
        lm = small.tile([1, NM], F32, tag="lm")
        nc.vector.tensor_tensor(lm[:], ls0, rt0[:], A.mult)
        nc.vector.tensor_tensor(lm[:], lm[:], v0[:], A.mult)
        ls = small.tile([1, 1], F32, tag="ls")
        nc.vector.tensor_reduce(ls[:], lm[:], mybir.AxisListType.X, A.add)
        nvm1 = small.tile([1, 1], F32, tag="nvm1")
        nc.vector.tensor_scalar(nvm1[:], nv0[:], 1.0, None, A.max)
        rnv = small.tile([1, 1], F32, tag="rnv")
        nc.vector.reciprocal(rnv[:], nvm1[:])
        agg = small.tile([1, 1], F32, tag="agg")
        nc.vector.tensor_tensor(agg[:], ls[:], rnv[:], A.mult)

        # dis = (nv > 1) * 0.5 * sum(lp * pair) / max(nv*(nv-1), 1)
        pm = small.tile([1, NP], F32, tag="pm")
        nc.vector.tensor_tensor(
            pm[:].rearrange("p (m n) -> p m n", m=NM),
            v0[:].unsqueeze(2).broadcast_to([1, NM, NM]),
            v0[:].unsqueeze(1).broadcast_to([1, NM, NM]),
            A.mult)
        nc.vector.tensor_tensor(pm[:], pm[:], ne_s[:], A.mult)
        nc.vector.tensor_tensor(pm[:], pm[:], lp[:], A.mult)
        sp = small.tile([1, 1], F32, tag="sp")
        nc.vector.tensor_reduce(sp[:], pm[:], mybir.AxisListType.X, A.add)
        pr_ = small.tile([1, 1], F32, tag="pr_")
        nc.vector.tensor_scalar(pr_[:], nv0[:], 1.0, None, A.subtract)
        nc.vector.tensor_tensor(pr_[:], pr_[:], nv0[:], A.mult)
        nc.vector.tensor_scalar(pr_[:], pr_[:], 1.0, None, A.max)
        rpr = small.tile([1, 1], F32, tag="rpr")
        nc.vector.reciprocal(rpr[:], pr_[:])
        dis = small.tile([1, 1], F32, tag="dis")
        nc.vector.tensor_tensor(dis[:], sp[:], rpr[:], A.mult)
        nc.vector.tensor_scalar(dis[:], dis[:], 0.5, None, A.mult)
        gate = small.tile([1, 1], F32, tag="gate")
        nc.vector.tensor_scalar(gate[:], nv0[:], 1.0, None, A.is_gt)
        nc.vector.tensor_tensor(dis[:], dis[:], gate[:], A.mult)

        # ---------- output ----------
        outt = small.tile([1, 2], F32, tag="outt")
        nc.vector.tensor_copy(outt[0:1, 0:1], agg[:])
        nc.vector.tensor_copy(outt[0:1, 1:2], dis[:])
        nc.sync.dma_start(out_ap, outt[:])


def build_nc(num_devices=8):
    nc = bacc.Bacc("TRN2", target_bir_lowering=False, debug=False,
                   num_devices=num_devices)
    sim = nc.dram_tensor("sim", (4, P), F32, kind="ExternalInput")
    tgt = nc.dram_tensor("tgt", (2, P), I32, kind="ExternalInput")
    ne = nc.dram_tensor("ne", (1, NM * NM), F32, kind="ExternalInput")
    miotas = nc.dram_tensor("miotas", (PARTS, 4), F32, kind="ExternalInput")
    bmask = nc.dram_tensor("bmask", (PARTS, PARTS), F32,
                           kind="ExternalInput")
    out = nc.dram_tensor("out", (1, 2), F32, kind="ExternalOutput")
    with tile.TileContext(nc) as tc:
        build_kernel_body(tc, out.ap(), sim.ap(), tgt.ap(), ne.ap(), miotas.ap(), bmask.ap())
    nc.compile()
    return nc


_NC_CACHE = {}


def _ne_const():
    return (1.0 - np.eye(NM, dtype=np.float32)).reshape(1, NM * NM)


def _miotas_const():
    return (np.arange(PARTS)[:, None] % 4 +
            4 * np.arange(4)[None, :]).astype(np.float32)


def _bmask_const():
    bm = np.zeros((PARTS, PARTS), np.float32)
    for g in range(32):
        bm[4 * g:4 * (g + 1), 4 * g:4 * (g + 1)] = 1.0
    return bm


def _get_exec(n_cores):
    """Build the Bass program and a cached jit-compiled SPMD executable."""
    if "fn" in _NC_CACHE:
        return _NC_CACHE
    import jax
    from jax.experimental.shard_map import shard_map
    from jax.sharding import Mesh, PartitionSpec
    from concourse import bass2jax

    bass2jax.install_neuronx_cc_hook()
    nc = build_nc(num_devices=n_cores)

    in_names = []
    out_names = []
    out_avals = []
    zero_outs = []
    for alloc in nc.m.functions[0].allocations:
        if not isinstance(alloc, mybir.MemoryLocationSet):
            continue
        name = alloc.memorylocations[0].name
        if alloc.kind == "ExternalInput":
            if nc.partition_id_tensor is not None and \
                    name == nc.partition_id_tensor.name:
                continue
            in_names.append(name)
        elif alloc.kind == "ExternalOutput":
            shape = tuple(alloc.tensor_shape)
            dtype = mybir.dt.np(alloc.dtype)
            out_names.append(name)
            out_avals.append(jax.core.ShapedArray(shape, dtype))
            zero_outs.append(np.zeros(shape, dtype))
    n_params = len(in_names)
    all_in_names = in_names + out_names
    partition_name = (nc.partition_id_tensor.name
                      if nc.partition_id_tensor is not None else None)
    if partition_name is not None:
        all_in_names = all_in_names + [partition_name]

    def _body(*args):
        operands = list(args)
        if partition_name is not None:
            operands.append(bass2jax.partition_id_tensor())
        outs = bass2jax._bass_exec_p.bind(
            *operands,
            out_avals=tuple(out_avals),
            in_names=tuple(all_in_names),
            out_names=tuple(out_names),
            lowering_input_output_aliases=(),
            sim_require_finite=True,
            sim_require_nnan=True,
            nc=nc,
        )
        return tuple(outs)

    devices = jax.devices()[:n_cores]
    mesh = Mesh(np.asarray(devices), ("core",))
    n_outs = len(out_names)
    fn = jax.jit(
        shard_map(
            _body, mesh=mesh,
            in_specs=(PartitionSpec("core"),) * (n_params + n_outs),
            out_specs=(PartitionSpec("core"),) * n_outs,
            check_rep=False,
        ),
        donate_argnums=tuple(range(n_params, n_params + n_outs)),
        keep_unused=True,
    )
    _NC_CACHE.update(dict(nc=nc, fn=fn, in_names=in_names,
                          out_names=out_names, zero_outs=zero_outs,
                          n_cores=n_cores))
    return _NC_CACHE


def prepare_inputs(preds, targets, n):
    """Concatenated per-core global inputs keyed by dram-parameter name."""
    sim = np.ascontiguousarray(
        preds[:, 2:6].reshape(n * 4, P).astype(np.float32, copy=False))
    tgt = np.ascontiguousarray(
        targets.reshape(n * 2, P).astype(np.int32, copy=False))
    ne = np.tile(_ne_const(), (n, 1))
    miotas = np.tile(_miotas_const(), (n, 1))
    bmask = np.tile(_bmask_const(), (n, 1))
    return {"sim": sim, "tgt": tgt, "ne": ne, "miotas": miotas,
            "bmask": bmask}


def run_prepared(exe, global_ins):
    args = [global_ins[k] for k in exe["in_names"]]
    zeros = [np.zeros((exe["n_cores"] * z.shape[0], *z.shape[1:]), z.dtype)
             for z in exe["zero_outs"]]
    out_arrs = exe["fn"](*args, *zeros)
    return [np.asarray(o) for o in out_arrs]


def kernel(preds: np.ndarray, targets: np.ndarray):
    n = preds.shape[0]
    assert preds.shape == (n, 6, H, W) and targets.shape == (n, 2, H, W)
    exe = _get_exec(n)
    outs = run_prepared(exe, prepare_inputs(preds, targets, n))
    out = outs[exe["out_names"].index("out")].reshape(n, 2)
    return out[:, 0].copy(), out[:, 1].copy()


# revision 97
# speedup vs baseline: 1.0152x; 1.0152x over previous
"""AggregationDiscriminationLoss kernel for 8 TRN2 NeuronCores.

Data-parallel over batch N=8 (one sample per core). Per core, pixels live in
[128, 3200] bf16 planes (P = 640*640):

- Segment sums (G[m,c], cnt_k): per-column PE matmuls — stationary =
  [4 sim channels + ones] at column q, moving = 15 one-hot columns of the
  kern ids (generated by DVE 4x eq-passes), PSUM-accumulated over all q.
- Gather G[text[p]]: 32-row slabs x 4 m-passes. Text ids are replicated 4x
  across partition sub-slots via a DRAM bounce, compared against per-
  partition m values (one DVE 4x pass per m-pass), then multiplied by a
  block-diagonal G table on the PE, accumulating all 16 ids in PSUM; the
  result is scattered back to pixel layout by DMA.
- Per-pixel loss chain on ACT (sqrt/relu/square/ln) + DVE diffs.
- Masked l-sums + cnt_t: per-column PE matmuls with a [l | ones] stationary,
  so one PSUM accumulation yields both, already reduced over all partitions.
- Pairwise-distance (dis) chain runs on partition 0 only, overlapping the
  gather; all final combines are partition-0 tinies.
Work is spread across DVE/ACT/Pool/PE; outputs (agg_i, dis_i) per core.
"""

import numpy as np

import concourse.bacc as bacc
import concourse.mybir as mybir
import concourse.tile as tile
from concourse import bass_utils

F32 = mybir.dt.float32
BF16 = mybir.dt.bfloat16
I32 = mybir.dt.int32
A = mybir.AluOpType
ACTF = mybir.ActivationFunctionType

M = 16
DELTA_AGG = 0.5
DELTA_DIS = 3.0
H = W = 640
P = H * W            # 409600
PARTS = 128
FREE = P // PARTS    # 3200
NM = M - 1           # ids 1..15 (id 0 never contributes to the losses)
GSPLIT = 1           # column splits of the gather pipeline

# engine-routing knobs, tuned against the TimelineSim cost model.
CFG = {
    "oh_pool_every": 4,   # every k-th kern one-hot pass on GpSimd
    "oht_pool_every": 99,  # every k-th text one-hot pass on GpSimd
}


def build_kernel_body(tc, out_ap, sim_ap, tgt_ap, ne_ap, miotas_ap, bmask_ap):
    nc = tc.nc

    simr = sim_ap.rearrange("c (p f) -> c p f", p=PARTS)   # (4, 128, 3200)
    tgtr = tgt_ap.rearrange("c (p f) -> c p f", p=PARTS)   # (2, 128, 3200)

    with tc.tile_pool(name="big", bufs=1) as big, \
         tc.tile_pool(name="stage", bufs=5) as stagep, \
         tc.tile_pool(name="ohp", bufs=2) as ohpool, \
         tc.tile_pool(name="kl", bufs=1) as klp, \
         tc.tile_pool(name="reps", bufs=2) as repp, \
         tc.tile_pool(name="stg", bufs=2) as stgp, \
         tc.tile_pool(name="ohs", bufs=2) as ohsp, \
         tc.tile_pool(name="gps", bufs=4, space="PSUM") as gpsp, \
         tc.tile_pool(name="dram", bufs=1, space="DRAM") as dramp, \
         tc.tile_pool(name="psum", bufs=1, space="PSUM") as psp, \
         tc.tile_pool(name="small", bufs=1) as small:

        NCH = 5
        CF = FREE // NCH

        # ---------- chunked loads + casts (kern first so the PE pipeline
        # can start as soon as chunk 0 of scv/oh is ready) ----------
        K16 = klp.tile([PARTS, FREE], BF16, tag="kl", name="K16")
        T16 = big.tile([PARTS, FREE], BF16, tag="T16")
        scv = big.tile([PARTS, 5, FREE], BF16, tag="scv")
        sc = [scv[:, c, :] for c in range(4)]
        nc.gpsimd.memset(scv[:, 4, :], 1.0)
        mio = small.tile([PARTS, 4], F32, tag="mio")
        nc.sync.dma_start(mio[:], miotas_ap)
        bmf = small.tile([PARTS, PARTS], F32, tag="bmf")
        nc.sync.dma_start(bmf[:], bmask_ap)
        bm16 = small.tile([PARTS, PARTS], BF16, tag="bm16")
        nc.vector.tensor_copy(bm16[:], bmf[:])

        tdr = dramp.tile([PARTS, FREE], BF16, tag="tdr")
        for ch in range(NCH):
            q0 = ch * CF
            k_stage = stagep.tile([PARTS, CF], I32, tag="stage", name="kst")
            nc.sync.dma_start(k_stage[:], tgtr[1][:, q0:q0 + CF])
            nc.gpsimd.tensor_copy(K16[:, q0:q0 + CF], k_stage[:])
            for c in range(4):
                s_stage = stagep.tile([PARTS, CF], F32, tag="stage",
                                      name="sst")
                nc.sync.dma_start(s_stage[:], simr[c][:, q0:q0 + CF])
                nc.scalar.copy(sc[c][:, q0:q0 + CF], s_stage[:])
        for ch in range(NCH):
            q0 = ch * CF
            t_stage = stagep.tile([PARTS, CF], I32, tag="stage", name="tst")
            nc.sync.dma_start(t_stage[:], tgtr[0][:, q0:q0 + CF])
            nc.scalar.copy(T16[:, q0:q0 + CF], t_stage[:])

        nc.sync.dma_start(tdr[:], T16[:])
        tdrv = tdr[:].rearrange("(t g) q -> t g q", g=32)

        # bias constants for ACT ops
        bm_agg = small.tile([PARTS, 1], F32, tag="bm_agg")
        nc.gpsimd.memset(bm_agg[:], -DELTA_AGG)
        bm_dis = small.tile([PARTS, 1], F32, tag="bm_dis")
        nc.gpsimd.memset(bm_dis[:], DELTA_DIS)

        # engine load balancer (estimated busy ns per engine); each option is
        # a dict engine -> added busy ns, chosen to minimize the peak load.
        load = {"dve": 0.0, "act": 0.0, "pool": 0.0}

        def pick(options):
            def peak(opt):
                return max(load[e] + opt.get(e, 0.0) for e in load)
            name = min(options, key=lambda k: peak(options[k]))
            for e, v in options[name].items():
                load[e] += v
            return name

        # ---------- phase A ----------
        # cnt_k / cnt_t via DVE 4x eq-passes (accum riding); G sums on the PE
        # as per-column matmuls: lhsT = scv[:, :, q] (4 sim channels), rhs =
        # one-hot of kern ids (15 m-columns), accumulated in PSUM over all q.
        ps = psp.tile([5, NM], F32, tag="ps")
        qg = 0
        for ch in range(NCH):
            q0 = ch * CF
            oh = ohpool.tile([PARTS, NM, CF], BF16, tag="oh", name="oh")
            for m in range(1, M):
                eng = "pool" if m % CFG["oh_pool_every"] == 0 else "dve"
                if eng == "pool":
                    nc.gpsimd.tensor_scalar(
                        oh[:, m - 1, :], K16[:, q0:q0 + CF], float(m), None,
                        A.is_equal)
                else:
                    nc.vector.tensor_scalar(
                        oh[:, m - 1, :], K16[:, q0:q0 + CF], float(m), None,
                        A.is_equal)
            for q in range(CF):
                nc.tensor.matmul(
                    ps[:], scv[:, :, q0 + q:q0 + q + 1], oh[:, :, q:q + 1],
                    start=(qg == 0), stop=(qg == FREE - 1))
                qg += 1

        # raw G sums + cnt_k (ones row): PSUM -> SBUF -> one row -> broadcast
        pss = small.tile([5, NM], F32, tag="pss")
        nc.scalar.copy(pss[:], ps[:])
        g1row = small.tile([1, 5 * NM], F32, tag="g1row")
        nc.sync.dma_start(g1row[:], pss[:])
        # gather-table path first (row 0 only, no broadcast dependency)
        mk0 = small.tile([1, NM], F32, tag="mk0")
        nc.vector.tensor_scalar(mk0[:], g1row[0:1, 4 * NM:5 * NM], 1.0,
                                None, A.max)
        rk0 = small.tile([1, NM], F32, tag="rk0")
        nc.vector.reciprocal(rk0[:], mk0[:])
        g0 = small.tile([1, 4 * NM], F32, tag="g0")
        nc.vector.tensor_tensor(
            g0[:].rearrange("p (c m) -> p c m", c=4),
            g1row[0:1, 0:4 * NM].rearrange("p (c m) -> p c m", c=4),
            rk0[:].unsqueeze(1).broadcast_to([1, 4, NM]),
            A.mult)


        # ---------- phase C: gather G[text] on the PE ----------
        # 32-row slabs x 4 m-passes: text ids are replicated 4x across
        # partition sub-slots (DRAM bounce), compared against per-partition
        # m values (one 4x DVE pass per m-pass), and multiplied by a
        # block-diagonal G table on the tensor engine, accumulating all 16
        # ids in PSUM. Result is scattered back to pixel layout by DMA.
        # G table (m-major, m=0 zero) -> DRAM bounce
        gsb = small.tile([1, 64], BF16, tag="gsb")
        nc.vector.memset(gsb[:], 0.0)
        nc.vector.tensor_copy(
            gsb[0:1, 4:64].rearrange("p (m c) -> p m c", c=4),
            g0[:].rearrange("p (c m) -> p m c", c=4))
        gd = dramp.tile([1, 64], BF16, tag="gd")
        nc.sync.dma_start(gd[:], gsb[:])
        gdv = gd[:].rearrange("one (m c) -> one m c", c=4)
        gblks = []
        for mp in range(4):
            grow = small.tile([PARTS, 4], BF16, tag=f"grow{mp}",
                              name=f"grow{mp}")
            nc.sync.dma_start(
                grow[:],
                gdv[:, 4 * mp:4 * mp + 4, :].broadcast_to([32, 4, 4]))
            gb = small.tile([PARTS, PARTS], BF16, tag=f"gblk{mp}",
                            name=f"gblk{mp}")
            nc.vector.tensor_tensor(
                gb[:].rearrange("p (gp c) -> p gp c", c=4),
                grow[:].unsqueeze(1).broadcast_to([PARTS, 32, 4]),
                bm16[:].rearrange("p (gp c) -> p gp c", c=4),
                A.mult)
            gblks.append(gb)

        QG = 400
        GH = FREE // GSPLIT
        gtv = big.tile([PARTS, 4, FREE], BF16, tag="gtv")
        for gh in range(GSPLIT):
            h0 = gh * GH
            for t in range(4):
                rep = repp.tile([PARTS, GH], BF16, tag="rep", name="rep")
                nc.sync.dma_start(
                    rep[:], tdrv[t, :, h0:h0 + GH].unsqueeze(1)
                    .broadcast_to([32, 4, GH]))
                ohs = []
                for mp in range(4):
                    oh4 = ohsp.tile([PARTS, GH], BF16, tag=f"oh4_{mp}",
                                    name=f"oh4_{mp}")
                    nc.vector.tensor_scalar(
                        oh4[:], rep[:], mio[:, mp:mp + 1], None, A.is_equal)
                    load["dve"] += 500
                    ohs.append(oh4)
                stg = stgp.tile([PARTS, GH], BF16, tag="stg", name="stg")
                for qc in range(GH // QG):
                    q0 = qc * QG
                    psg = gpsp.tile([PARTS, QG], F32, tag="psg", name="psg")
                    for mp in range(4):
                        nc.tensor.matmul(
                            psg[:], gblks[mp][:], ohs[mp][:, q0:q0 + QG],
                            start=(mp == 0), stop=(mp == 3))
                    eng = pick({"dve": {"dve": 650}, "act": {"act": 450}})
                    if eng == "act":
                        nc.scalar.copy(stg[:, q0:q0 + QG], psg[:])
                    else:
                        nc.vector.tensor_copy(stg[:, q0:q0 + QG], psg[:])
                for j in range(4):
                    nc.sync.dma_start(
                        gtv[32 * t + 8 * j:32 * t + 8 * (j + 1), :,
                            h0:h0 + GH],
                        stg[32 * j:32 * (j + 1), :])
        gt = [gtv[:, c, :] for c in range(4)]

        # ---------- dis heavy part (needs only G; overlaps gather/tail) ----
        # forward-equivalent without the where(pair, sq, 1) guard: invalid
        # pairs produce finite values that are masked after the fact.
        NP = NM * NM
        ne_s = small.tile([1, NP], F32, tag="ne_s")
        nc.sync.dma_start(ne_s[:], ne_ap)
        dif = stgp.tile([1, NP * 4], F32, tag="stg", name="dif")
        g_m = g0[:].rearrange("p (c m) -> p m c", c=4).unsqueeze(2)
        g_mp = g0[:].rearrange("p (c m) -> p m c", c=4).unsqueeze(1)
        nc.vector.tensor_tensor(
            dif[:].rearrange("p (m n c) -> p m n c", m=NM, n=NM),
            g_m.broadcast_to([1, NM, NM, 4]),
            g_mp.broadcast_to([1, NM, NM, 4]),
            A.subtract)
        nc.vector.tensor_tensor(dif[:], dif[:], dif[:], A.mult)
        lp = small.tile([1, NP], F32, tag="lp")
        nc.vector.tensor_reduce(
            lp[:], dif[:].rearrange("p (n c) -> p n c", c=4),
            mybir.AxisListType.X, A.add)
        nc.scalar.activation(lp[:], lp[:], ACTF.Sqrt)
        nc.scalar.activation(lp[:], lp[:], ACTF.Relu, bias=bm_dis[0:1, :],
                             scale=-1.0)
        nc.vector.tensor_tensor(lp[:], lp[:], lp[:], A.mult)
        nc.scalar.activation(lp[:], lp[:], ACTF.Ln, bias=1.0)

        # ---------- tail loop 1: diff/sq/d2/sqrt/u/u2 per chunk ----------
        lpl = scv[:, 3, :]   # sim plane 3 is dead after its diff
        for ch in range(NCH):
            q0 = ch * CF
            s_ = slice(q0, q0 + CF)
            for c in range(4):
                nc.vector.tensor_tensor(gt[c][:, s_], sc[c][:, s_],
                                        gt[c][:, s_], A.subtract)
                nc.vector.tensor_tensor(gt[c][:, s_], gt[c][:, s_],
                                        gt[c][:, s_], A.mult)
                load["dve"] += 660
            nc.vector.tensor_tensor(gt[0][:, s_], gt[0][:, s_],
                                    gt[1][:, s_], A.add)
            nc.vector.tensor_tensor(gt[2][:, s_], gt[2][:, s_],
                                    gt[3][:, s_], A.add)
            nc.vector.tensor_tensor(gt[0][:, s_], gt[0][:, s_],
                                    gt[2][:, s_], A.add)  # d2
            load["dve"] += 3 * 330
            nc.scalar.activation(gt[1][:, s_], gt[0][:, s_], ACTF.Sqrt)
            nc.scalar.activation(gt[2][:, s_], gt[1][:, s_], ACTF.Relu,
                                 bias=bm_agg[:])                     # u
            nc.scalar.activation(gt[3][:, s_], gt[2][:, s_], ACTF.Square)
            load["act"] += 3 * 700

        # ---------- tail loop 2: ln + one-hots + PE masked-l sums ----------
        # stationary carries [l | ones] so the same matmuls also produce
        # cnt_t globally (no partition reduction needed).
        oht_tiles = []
        for ch in range(NCH):
            q0 = ch * CF
            s_ = slice(q0, q0 + CF)
            nc.scalar.activation(lpl[:, s_], gt[3][:, s_], ACTF.Ln,
                                 bias=1.0)
            load["act"] += 700
            oht = ohpool.tile([PARTS, NM, CF], BF16, tag="oh", name="oht")
            for m in range(1, M):
                nc.vector.tensor_scalar(
                    oht[:, m - 1, :], T16[:, s_], float(m), None,
                    A.is_equal)
            oht_tiles.append(oht)
        ps2 = psp.tile([2, NM], F32, tag="ps2")
        qg = 0
        for ch in range(NCH):
            q0 = ch * CF
            oht = oht_tiles[ch]
            for q in range(CF):
                nc.tensor.matmul(
                    ps2[:], scv[:, 3:5, q0 + q:q0 + q + 1],
                    oht[:, :, q:q + 1],
                    start=(qg == 0), stop=(qg == FREE - 1))
                qg += 1
        lred = small.tile([2, NM], F32, tag="lred")
        nc.vector.tensor_copy(lred[:], ps2[:])
        l1row = small.tile([1, 2 * NM], F32, tag="l1row")
        nc.sync.dma_start(l1row[:], lred[:])

        # ---------- final combines, all on partition 0 ----------
        ck0 = g1row[0:1, 4 * NM:5 * NM]
        ls0 = l1row[0:1, 0:NM]
        ct0 = l1row[0:1, NM:2 * NM]
        mt0 = small.tile([1, NM], F32, tag="mt0")
        nc.vector.tensor_scalar(mt0[:], ct0, 1.0, None, A.max)
        rt0 = small.tile([1, NM], F32, tag="rt0")
        nc.vector.reciprocal(rt0[:], mt0[:])
        vk0 = small.tile([1, NM], F32, tag="vk0")
        nc.vector.tensor_scalar(vk0[:], ck0, 0.0, None, A.is_gt)
        v0 = small.tile([1, NM], F32, tag="v0")
        nc.vector.tensor_scalar(v0[:], ct0, 0.0, None, A.is_gt)
        nc.vector.tensor_tensor(v0[:], v0[:], vk0[:], A.mult)
        nv0 = small.tile([1, 1], F32, tag="nv0")
        nc.vector.tensor_reduce(nv0[:], v0[:], mybir.AxisListType.X, A.add)

        # agg = sum(valid * l_sum / max(cnt_t,1)) / max(nv, 1)
        lm = small.tile([1, NM], F32, tag="lm")
        nc.vector.tensor_tensor(lm[:], ls0, rt0[:], A.mult)
        nc.vector.tensor_tensor(lm[:], lm[:], v0[:], A.mult)
        ls = small.tile([1, 1], F32, tag="ls")
        nc.vector.tensor_reduce(ls[:], lm[:], mybir.AxisListType.X, A.add)
        nvm1 = small.tile([1, 1], F32, tag="nvm1")
        nc.vector.tensor_scalar(nvm1[:], nv0[:], 1.0, None, A.max)
        rnv = small.tile([1, 1], F32, tag="rnv")
        nc.vector.reciprocal(rnv[:], nvm1[:])
        agg = small.tile([1, 1], F32, tag="agg")
        nc.vector.tensor_tensor(agg[:], ls[:], rnv[:], A.mult)

        # dis = (nv > 1) * 0.5 * sum(lp * pair) / max(nv*(nv-1), 1)
        pm = small.tile([1, NP], F32, tag="pm")
        nc.vector.tensor_tensor(
            pm[:].rearrange("p (m n) -> p m n", m=NM),
            v0[:].unsqueeze(2).broadcast_to([1, NM, NM]),
            v0[:].unsqueeze(1).broadcast_to([1, NM, NM]),
            A.mult)
        nc.vector.tensor_tensor(pm[:], pm[:], ne_s[:], A.mult)
        nc.vector.tensor_tensor(pm[:], pm[:], lp[:], A.mult)
        sp = small.tile([1, 1], F32, tag="sp")
        nc.vector.tensor_reduce(sp[:], pm[:], mybir.AxisListType.X, A.add)
        pr_ = small.tile([1, 1], F32, tag="pr_")
        nc.vector.tensor_scalar(pr_[:], nv0[:], 1.0, None, A.subtract)
        nc.vector.tensor_tensor(pr_[:], pr_[:], nv0[:], A.mult)
        nc.vector.tensor_scalar(pr_[:], pr_[:], 1.0, None, A.max)
        rpr = small.tile([1, 1], F32, tag="rpr")
        nc.vector.reciprocal(rpr[:], pr_[:])
        dis = small.tile([1, 1], F32, tag="dis")
        nc.vector.tensor_tensor(dis[:], sp[:], rpr[:], A.mult)
        nc.vector.tensor_scalar(dis[:], dis[:], 0.5, None, A.mult)
        gate = small.tile([1, 1], F32, tag="gate")
        nc.vector.tensor_scalar(gate[:], nv0[:], 1.0, None, A.is_gt)
        nc.vector.tensor_tensor(dis[:], dis[:], gate[:], A.mult)

        # ---------- output ----------
        outt = small.tile([1, 2], F32, tag="outt")
        nc.vector.tensor_copy(outt[0:1, 0:1], agg[:])
        nc.vector.tensor_copy(outt[0:1, 1:2], dis[:])
        nc.sync.dma_start(out_ap, outt[:])


def build_nc(num_devices=8):
    nc = bacc.Bacc("TRN2", target_bir_lowering=False, debug=False,
                   num_devices=num_devices)
    sim = nc.dram_tensor("sim", (4, P), F32, kind="ExternalInput")
    tgt = nc.dram_tensor("tgt", (2, P), I32, kind="ExternalInput")
    ne = nc.dram_tensor("ne", (1, NM * NM), F32, kind="ExternalInput")
    miotas = nc.dram_tensor("miotas", (PARTS, 4), F32, kind="ExternalInput")
    bmask = nc.dram_tensor("bmask", (PARTS, PARTS), F32,
                           kind="ExternalInput")
    out = nc.dram_tensor("out", (1, 2), F32, kind="ExternalOutput")
    with tile.TileContext(nc) as tc:
        build_kernel_body(tc, out.ap(), sim.ap(), tgt.ap(), ne.ap(), miotas.ap(), bmask.ap())
    nc.compile()
    return nc


_NC_CACHE = {}


def _ne_const():
    return (1.0 - np.eye(NM, dtype=np.float32)).reshape(1, NM * NM)


def _miotas_const():
    return (np.arange(PARTS)[:, None] % 4 +
            4 * np.arange(4)[None, :]).astype(np.float32)


def _bmask_const():
    bm = np.zeros((PARTS, PARTS), np.float32)
    for g in range(32):
        bm[4 * g:4 * (g + 1), 4 * g:4 * (g + 1)] = 1.0
    return bm


def _get_exec(n_cores):
    """Build the Bass program and a cached jit-compiled SPMD executable."""
    if "fn" in _NC_CACHE:
        return _NC_CACHE
    import jax
    from jax.experimental.shard_map import shard_map
    from jax.sharding import Mesh, PartitionSpec
    from concourse import bass2jax

    bass2jax.install_neuronx_cc_hook()
    nc = build_nc(num_devices=n_cores)

    in_names = []
    out_names = []
    out_avals = []
    zero_outs = []
    for alloc in nc.m.functions[0].allocations:
        if not isinstance(alloc, mybir.MemoryLocationSet):
            continue
        name = alloc.memorylocations[0].name
        if alloc.kind == "ExternalInput":
            if nc.partition_id_tensor is not None and \
                    name == nc.partition_id_tensor.name:
                continue
            in_names.append(name)
        elif alloc.kind == "ExternalOutput":
            shape = tuple(alloc.tensor_shape)
            dtype = mybir.dt.np(alloc.dtype)
            out_names.append(name)
            out_avals.append(jax.core.ShapedArray(shape, dtype))
            zero_outs.append(np.zeros(shape, dtype))
    n_params = len(in_names)
    all_in_names = in_names + out_names
    partition_name = (nc.partition_id_tensor.name
                      if nc.partition_id_tensor is not None else None)
    if partition_name is not None:
        all_in_names = all_in_names + [partition_name]

    def _body(*args):
        operands = list(args)
        if partition_name is not None:
            operands.append(bass2jax.partition_id_tensor())
        outs = bass2jax._bass_exec_p.bind(
            *operands,
            out_avals=tuple(out_avals),
            in_names=tuple(all_in_names),
            out_names=tuple(out_names),
            lowering_input_output_aliases=(),
            sim_require_finite=True,
            sim_require_nnan=True,
            nc=nc,
        )
        return tuple(outs)

    devices = jax.devices()[:n_cores]
    mesh = Mesh(np.asarray(devices), ("core",))
    n_outs = len(out_names)
    fn = jax.jit(
        shard_map(
            _body, mesh=mesh,
            in_specs=(PartitionSpec("core"),) * (n_params + n_outs),
            out_specs=(PartitionSpec("core"),) * n_outs,
            check_rep=False,
        ),
        donate_argnums=tuple(range(n_params, n_params + n_outs)),
        keep_unused=True,
    )
    _NC_CACHE.update(dict(nc=nc, fn=fn, in_names=in_names,
                          out_names=out_names, zero_outs=zero_outs,
                          n_cores=n_cores))
    return _NC_CACHE


def prepare_inputs(preds, targets, n):
    """Concatenated per-core global inputs keyed by dram-parameter name."""
    sim = np.ascontiguousarray(
        preds[:, 2:6].reshape(n * 4, P).astype(np.float32, copy=False))
    tgt = np.ascontiguousarray(
        targets.reshape(n * 2, P).astype(np.int32, copy=False))
    ne = np.tile(_ne_const(), (n, 1))
    miotas = np.tile(_miotas_const(), (n, 1))
    bmask = np.tile(_bmask_const(), (n, 1))
    return {"sim": sim, "tgt": tgt, "ne": ne, "miotas": miotas,
            "bmask": bmask}


def run_prepared(exe, global_ins):
    args = [global_ins[k] for k in exe["in_names"]]
    zeros = [np.zeros((exe["n_cores"] * z.shape[0], *z.shape[1:]), z.dtype)
             for z in exe["zero_outs"]]
    out_arrs = exe["fn"](*args, *zeros)
    return [np.asarray(o) for o in out_arrs]


def kernel(preds: np.ndarray, targets: np.ndarray):
    n = preds.shape[0]
    assert preds.shape == (n, 6, H, W) and targets.shape == (n, 2, H, W)
    exe = _get_exec(n)
    outs = run_prepared(exe, prepare_inputs(preds, targets, n))
    out = outs[exe["out_names"].index("out")].reshape(n, 2)
    return out[:, 0].copy(), out[:, 1].copy()


# revision 102
# speedup vs baseline: 1.0153x; 1.0001x over previous
"""AggregationDiscriminationLoss kernel for 8 TRN2 NeuronCores.

Data-parallel over batch N=8 (one sample per core). Per core, pixels live in
[128, 3200] bf16 planes (P = 640*640):

- Segment sums (G[m,c], cnt_k): per-column PE matmuls — stationary =
  [4 sim channels + ones] at column q, moving = 15 one-hot columns of the
  kern ids (generated by DVE 4x eq-passes), PSUM-accumulated over all q.
- Gather G[text[p]]: 32-row slabs x 4 m-passes. Text ids are replicated 4x
  across partition sub-slots via a DRAM bounce, compared against per-
  partition m values (one DVE 4x pass per m-pass), then multiplied by a
  block-diagonal G table on the PE, accumulating all 16 ids in PSUM; the
  result is scattered back to pixel layout by DMA.
- Per-pixel loss chain on ACT (sqrt/relu/square/ln) + DVE diffs.
- Masked l-sums + cnt_t: per-column PE matmuls with a [l | ones] stationary,
  so one PSUM accumulation yields both, already reduced over all partitions.
- Pairwise-distance (dis) chain runs on partition 0 only, overlapping the
  gather; all final combines are partition-0 tinies.
Work is spread across DVE/ACT/Pool/PE; outputs (agg_i, dis_i) per core.
"""

import numpy as np

import concourse.bacc as bacc
import concourse.mybir as mybir
import concourse.tile as tile
from concourse import bass_utils

F32 = mybir.dt.float32
BF16 = mybir.dt.bfloat16
I32 = mybir.dt.int32
A = mybir.AluOpType
ACTF = mybir.ActivationFunctionType

M = 16
DELTA_AGG = 0.5
DELTA_DIS = 3.0
H = W = 640
P = H * W            # 409600
PARTS = 128
FREE = P // PARTS    # 3200
NM = M - 1           # ids 1..15 (id 0 never contributes to the losses)
GSPLIT = 1           # column splits of the gather pipeline

# engine-routing knobs, tuned against the TimelineSim cost model.
CFG = {
    "oh_pool_every": 4,   # every k-th kern one-hot pass on GpSimd
    "oht_pool_every": 99,  # every k-th text one-hot pass on GpSimd
}


def build_kernel_body(tc, out_ap, sim_ap, tgt_ap, ne_ap, miotas_ap, bmask_ap):
    nc = tc.nc

    simr = sim_ap.rearrange("c (p f) -> c p f", p=PARTS)   # (4, 128, 3200)
    tgtr = tgt_ap.rearrange("c (p f) -> c p f", p=PARTS)   # (2, 128, 3200)

    with tc.tile_pool(name="big", bufs=1) as big, \
         tc.tile_pool(name="stage", bufs=5) as stagep, \
         tc.tile_pool(name="ohp", bufs=2) as ohpool, \
         tc.tile_pool(name="kl", bufs=1) as klp, \
         tc.tile_pool(name="reps", bufs=2) as repp, \
         tc.tile_pool(name="stg", bufs=2) as stgp, \
         tc.tile_pool(name="ohs", bufs=2) as ohsp, \
         tc.tile_pool(name="gps", bufs=4, space="PSUM") as gpsp, \
         tc.tile_pool(name="dram", bufs=1, space="DRAM") as dramp, \
         tc.tile_pool(name="psum", bufs=1, space="PSUM") as psp, \
         tc.tile_pool(name="small", bufs=1) as small:

        NCH = 5
        CF = FREE // NCH

        # ---------- chunked loads + casts (kern first so the PE pipeline
        # can start as soon as chunk 0 of scv/oh is ready) ----------
        K16 = klp.tile([PARTS, FREE], BF16, tag="kl", name="K16")
        T16 = big.tile([PARTS, FREE], BF16, tag="T16")
        scv = big.tile([PARTS, 5, FREE], BF16, tag="scv")
        sc = [scv[:, c, :] for c in range(4)]
        nc.gpsimd.memset(scv[:, 4, :], 1.0)
        mio = small.tile([PARTS, 4], F32, tag="mio")
        nc.sync.dma_start(mio[:], miotas_ap)
        bmf = small.tile([PARTS, PARTS], F32, tag="bmf")
        nc.sync.dma_start(bmf[:], bmask_ap)
        bm16 = small.tile([PARTS, PARTS], BF16, tag="bm16")
        nc.vector.tensor_copy(bm16[:], bmf[:])

        tdr = dramp.tile([PARTS, FREE], BF16, tag="tdr")
        for ch in range(NCH):
            q0 = ch * CF
            k_stage = stagep.tile([PARTS, CF], I32, tag="stage", name="kst")
            nc.sync.dma_start(k_stage[:], tgtr[1][:, q0:q0 + CF])
            nc.gpsimd.tensor_copy(K16[:, q0:q0 + CF], k_stage[:])
            for c in range(4):
                s_stage = stagep.tile([PARTS, CF], F32, tag="stage",
                                      name="sst")
                nc.sync.dma_start(s_stage[:], simr[c][:, q0:q0 + CF])
                nc.scalar.copy(sc[c][:, q0:q0 + CF], s_stage[:])
        for ch in range(NCH):
            q0 = ch * CF
            t_stage = stagep.tile([PARTS, CF], I32, tag="stage", name="tst")
            nc.sync.dma_start(t_stage[:], tgtr[0][:, q0:q0 + CF])
            nc.scalar.copy(T16[:, q0:q0 + CF], t_stage[:])

        nc.sync.dma_start(tdr[:], T16[:])
        tdrv = tdr[:].rearrange("(t g) q -> t g q", g=32)

        # bias constants for ACT ops
        bm_agg = small.tile([PARTS, 1], F32, tag="bm_agg")
        nc.gpsimd.memset(bm_agg[:], -DELTA_AGG)
        bm_dis = small.tile([PARTS, 1], F32, tag="bm_dis")
        nc.gpsimd.memset(bm_dis[:], DELTA_DIS)

        # engine load balancer (estimated busy ns per engine); each option is
        # a dict engine -> added busy ns, chosen to minimize the peak load.
        load = {"dve": 0.0, "act": 0.0, "pool": 0.0}

        def pick(options):
            def peak(opt):
                return max(load[e] + opt.get(e, 0.0) for e in load)
            name = min(options, key=lambda k: peak(options[k]))
            for e, v in options[name].items():
                load[e] += v
            return name

        # ---------- phase A ----------
        # cnt_k / cnt_t via DVE 4x eq-passes (accum riding); G sums on the PE
        # as per-column matmuls: lhsT = scv[:, :, q] (4 sim channels), rhs =
        # one-hot of kern ids (15 m-columns), accumulated in PSUM over all q.
        ps = psp.tile([5, NM], F32, tag="ps")
        qg = 0
        for ch in range(NCH):
            q0 = ch * CF
            oh = ohpool.tile([PARTS, NM, CF], BF16, tag="oh", name="oh")
            for m in range(1, M):
                eng = "pool" if m % CFG["oh_pool_every"] == 0 else "dve"
                if eng == "pool":
                    nc.gpsimd.tensor_scalar(
                        oh[:, m - 1, :], K16[:, q0:q0 + CF], float(m), None,
                        A.is_equal)
                else:
                    nc.vector.tensor_scalar(
                        oh[:, m - 1, :], K16[:, q0:q0 + CF], float(m), None,
                        A.is_equal)
            for q in range(CF):
                nc.tensor.matmul(
                    ps[:], scv[:, :, q0 + q:q0 + q + 1], oh[:, :, q:q + 1],
                    start=(qg == 0), stop=(qg == FREE - 1))
                qg += 1

        # raw G sums + cnt_k (ones row): PSUM -> SBUF -> one row -> broadcast
        pss = small.tile([5, NM], F32, tag="pss")
        nc.scalar.copy(pss[:], ps[:])
        g1row = small.tile([1, 5 * NM], F32, tag="g1row")
        nc.sync.dma_start(g1row[:], pss[:])
        # gather-table path first (row 0 only, no broadcast dependency)
        mk0 = small.tile([1, NM], F32, tag="mk0")
        nc.vector.tensor_scalar(mk0[:], g1row[0:1, 4 * NM:5 * NM], 1.0,
                                None, A.max)
        rk0 = small.tile([1, NM], F32, tag="rk0")
        nc.vector.reciprocal(rk0[:], mk0[:])
        g0 = small.tile([1, 4 * NM], F32, tag="g0")
        nc.vector.tensor_tensor(
            g0[:].rearrange("p (c m) -> p c m", c=4),
            g1row[0:1, 0:4 * NM].rearrange("p (c m) -> p c m", c=4),
            rk0[:].unsqueeze(1).broadcast_to([1, 4, NM]),
            A.mult)


        # ---------- phase C: gather G[text] on the PE ----------
        # 32-row slabs x 4 m-passes: text ids are replicated 4x across
        # partition sub-slots (DRAM bounce), compared against per-partition
        # m values (one 4x DVE pass per m-pass), and multiplied by a
        # block-diagonal G table on the tensor engine, accumulating all 16
        # ids in PSUM. Result is scattered back to pixel layout by DMA.
        # G table (m-major, m=0 zero) -> DRAM bounce
        gsb = small.tile([1, 64], BF16, tag="gsb")
        nc.vector.memset(gsb[:], 0.0)
        nc.vector.tensor_copy(
            gsb[0:1, 4:64].rearrange("p (m c) -> p m c", c=4),
            g0[:].rearrange("p (c m) -> p m c", c=4))
        gd = dramp.tile([1, 64], BF16, tag="gd")
        nc.sync.dma_start(gd[:], gsb[:])
        gdv = gd[:].rearrange("one (m c) -> one m c", c=4)
        gblks = []
        for mp in range(4):
            grow = small.tile([PARTS, 4], BF16, tag=f"grow{mp}",
                              name=f"grow{mp}")
            nc.sync.dma_start(
                grow[:],
                gdv[:, 4 * mp:4 * mp + 4, :].broadcast_to([32, 4, 4]))
            gb = small.tile([PARTS, PARTS], BF16, tag=f"gblk{mp}",
                            name=f"gblk{mp}")
            nc.vector.tensor_tensor(
                gb[:].rearrange("p (gp c) -> p gp c", c=4),
                grow[:].unsqueeze(1).broadcast_to([PARTS, 32, 4]),
                bm16[:].rearrange("p (gp c) -> p gp c", c=4),
                A.mult)
            gblks.append(gb)

        QG = 400
        GH = FREE // GSPLIT
        gtv = big.tile([PARTS, 4, FREE], BF16, tag="gtv")
        for gh in range(GSPLIT):
            h0 = gh * GH
            for t in range(4):
                rep = repp.tile([PARTS, GH], BF16, tag="rep", name="rep")
                nc.sync.dma_start(
                    rep[:], tdrv[t, :, h0:h0 + GH].unsqueeze(1)
                    .broadcast_to([32, 4, GH]))
                ohs = []
                for mp in range(4):
                    oh4 = ohsp.tile([PARTS, GH], BF16, tag=f"oh4_{mp}",
                                    name=f"oh4_{mp}")
                    nc.vector.tensor_scalar(
                        oh4[:], rep[:], mio[:, mp:mp + 1], None, A.is_equal)
                    load["dve"] += 500
                    ohs.append(oh4)
                stg = stgp.tile([PARTS, GH], BF16, tag="stg", name="stg")
                for qc in range(GH // QG):
                    q0 = qc * QG
                    psg = gpsp.tile([PARTS, QG], F32, tag="psg", name="psg")
                    for mp in range(4):
                        nc.tensor.matmul(
                            psg[:], gblks[mp][:], ohs[mp][:, q0:q0 + QG],
                            start=(mp == 0), stop=(mp == 3))
                    nc.scalar.copy(stg[:, q0:q0 + QG], psg[:])
                for j in range(4):
                    nc.sync.dma_start(
                        gtv[32 * t + 8 * j:32 * t + 8 * (j + 1), :,
                            h0:h0 + GH],
                        stg[32 * j:32 * (j + 1), :])
        gt = [gtv[:, c, :] for c in range(4)]

        # ---------- dis heavy part (needs only G; overlaps gather/tail) ----
        # forward-equivalent without the where(pair, sq, 1) guard: invalid
        # pairs produce finite values that are masked after the fact.
        NP = NM * NM
        ne_s = small.tile([1, NP], F32, tag="ne_s")
        nc.sync.dma_start(ne_s[:], ne_ap)
        dif = stgp.tile([1, NP * 4], F32, tag="stg", name="dif")
        g_m = g0[:].rearrange("p (c m) -> p m c", c=4).unsqueeze(2)
        g_mp = g0[:].rearrange("p (c m) -> p m c", c=4).unsqueeze(1)
        nc.vector.tensor_tensor(
            dif[:].rearrange("p (m n c) -> p m n c", m=NM, n=NM),
            g_m.broadcast_to([1, NM, NM, 4]),
            g_mp.broadcast_to([1, NM, NM, 4]),
            A.subtract)
        nc.vector.tensor_tensor(dif[:], dif[:], dif[:], A.mult)
        lp = small.tile([1, NP], F32, tag="lp")
        nc.vector.tensor_reduce(
            lp[:], dif[:].rearrange("p (n c) -> p n c", c=4),
            mybir.AxisListType.X, A.add)
        nc.scalar.activation(lp[:], lp[:], ACTF.Sqrt)
        nc.scalar.activation(lp[:], lp[:], ACTF.Relu, bias=bm_dis[0:1, :],
                             scale=-1.0)
        nc.vector.tensor_tensor(lp[:], lp[:], lp[:], A.mult)
        nc.scalar.activation(lp[:], lp[:], ACTF.Ln, bias=1.0)

        # ---------- tail loop 1: diff/sq/d2/sqrt/u/u2 per chunk ----------
        lpl = scv[:, 3, :]   # sim plane 3 is dead after its diff
        for ch in range(NCH):
            q0 = ch * CF
            s_ = slice(q0, q0 + CF)
            for c in range(4):
                nc.vector.tensor_tensor(gt[c][:, s_], sc[c][:, s_],
                                        gt[c][:, s_], A.subtract)
                nc.vector.tensor_tensor(gt[c][:, s_], gt[c][:, s_],
                                        gt[c][:, s_], A.mult)
                load["dve"] += 660
            nc.vector.tensor_tensor(gt[0][:, s_], gt[0][:, s_],
                                    gt[1][:, s_], A.add)
            nc.vector.tensor_tensor(gt[2][:, s_], gt[2][:, s_],
                                    gt[3][:, s_], A.add)
            nc.vector.tensor_tensor(gt[0][:, s_], gt[0][:, s_],
                                    gt[2][:, s_], A.add)  # d2
            load["dve"] += 3 * 330
            nc.scalar.activation(gt[1][:, s_], gt[0][:, s_], ACTF.Sqrt)
            nc.scalar.activation(gt[2][:, s_], gt[1][:, s_], ACTF.Relu,
                                 bias=bm_agg[:])                     # u
            nc.scalar.activation(gt[3][:, s_], gt[2][:, s_], ACTF.Square)
            load["act"] += 3 * 700

        # ---------- tail loop 2: ln + one-hots + PE masked-l sums ----------
        # stationary carries [l | ones] so the same matmuls also produce
        # cnt_t globally (no partition reduction needed).
        oht_tiles = []
        for ch in range(NCH):
            q0 = ch * CF
            s_ = slice(q0, q0 + CF)
            nc.scalar.activation(lpl[:, s_], gt[3][:, s_], ACTF.Ln,
                                 bias=1.0)
            load["act"] += 700
            oht = ohpool.tile([PARTS, NM, CF], BF16, tag="oh", name="oht")
            for m in range(1, M):
                nc.vector.tensor_scalar(
                    oht[:, m - 1, :], T16[:, s_], float(m), None,
                    A.is_equal)
            oht_tiles.append(oht)
        ps2 = psp.tile([2, NM], F32, tag="ps2")
        qg = 0
        for ch in range(NCH):
            q0 = ch * CF
            oht = oht_tiles[ch]
            for q in range(CF):
                nc.tensor.matmul(
                    ps2[:], scv[:, 3:5, q0 + q:q0 + q + 1],
                    oht[:, :, q:q + 1],
                    start=(qg == 0), stop=(qg == FREE - 1))
                qg += 1
        lred = small.tile([2, NM], F32, tag="lred")
        nc.vector.tensor_copy(lred[:], ps2[:])
        l1row = small.tile([1, 2 * NM], F32, tag="l1row")
        nc.sync.dma_start(l1row[:], lred[:])

        # ---------- final combines, all on partition 0 ----------
        ck0 = g1row[0:1, 4 * NM:5 * NM]
        ls0 = l1row[0:1, 0:NM]
        ct0 = l1row[0:1, NM:2 * NM]
        mt0 = small.tile([1, NM], F32, tag="mt0")
        nc.vector.tensor_scalar(mt0[:], ct0, 1.0, None, A.max)
        rt0 = small.tile([1, NM], F32, tag="rt0")
        nc.vector.reciprocal(rt0[:], mt0[:])
        vk0 = small.tile([1, NM], F32, tag="vk0")
        nc.vector.tensor_scalar(vk0[:], ck0, 0.0, None, A.is_gt)
        v0 = small.tile([1, NM], F32, tag="v0")
        nc.vector.tensor_scalar(v0[:], ct0, 0.0, None, A.is_gt)
        nc.vector.tensor_tensor(v0[:], v0[:], vk0[:], A.mult)
        nv0 = small.tile([1, 1], F32, tag="nv0")
        nc.vector.tensor_reduce(nv0[:], v0[:], mybir.AxisListType.X, A.add)

        # agg = sum(valid * l_sum / max(cnt_t,1)) / max(nv, 1)
        lm = small.tile([1, NM], F32, tag="lm")
        nc.vector.tensor_tensor(lm[:], ls0, rt0[:], A.mult)
        nc.vector.tensor_tensor(lm[:], lm[:], v0[:], A.mult)
        ls = small.tile([1, 1], F32, tag="ls")
        nc.vector.tensor_reduce(ls[:], lm[:], mybir.AxisListType.X, A.add)
        nvm1 = small.tile([1, 1], F32, tag="nvm1")
        nc.vector.tensor_scalar(nvm1[:], nv0[:], 1.0, None, A.max)
        rnv = small.tile([1, 1], F32, tag="rnv")
        nc.vector.reciprocal(rnv[:], nvm1[:])
        agg = small.tile([1, 1], F32, tag="agg")
        nc.vector.tensor_tensor(agg[:], ls[:], rnv[:], A.mult)

        # dis = (nv > 1) * 0.5 * sum(lp * pair) / max(nv*(nv-1), 1)
        pm = small.tile([1, NP], F32, tag="pm")
        nc.vector.tensor_tensor(
            pm[:].rearrange("p (m n) -> p m n", m=NM),
            v0[:].unsqueeze(2).broadcast_to([1, NM, NM]),
            v0[:].unsqueeze(1).broadcast_to([1, NM, NM]),
            A.mult)
        nc.vector.tensor_tensor(pm[:], pm[:], ne_s[:], A.mult)
        nc.vector.tensor_tensor(pm[:], pm[:], lp[:], A.mult)
        sp = small.tile([1, 1], F32, tag="sp")
        nc.vector.tensor_reduce(sp[:], pm[:], mybir.AxisListType.X, A.add)
        pr_ = small.tile([1, 1], F32, tag="pr_")
        nc.vector.tensor_scalar(pr_[:], nv0[:], 1.0, None, A.subtract)
        nc.vector.tensor_tensor(pr_[:], pr_[:], nv0[:], A.mult)
        nc.vector.tensor_scalar(pr_[:], pr_[:], 1.0, None, A.max)
        rpr = small.tile([1, 1], F32, tag="rpr")
        nc.vector.reciprocal(rpr[:], pr_[:])
        dis = small.tile([1, 1], F32, tag="dis")
        nc.vector.tensor_tensor(dis[:], sp[:], rpr[:], A.mult)
        nc.vector.tensor_scalar(dis[:], dis[:], 0.5, None, A.mult)
        gate = small.tile([1, 1], F32, tag="gate")
        nc.vector.tensor_scalar(gate[:], nv0[:], 1.0, None, A.is_gt)
        nc.vector.tensor_tensor(dis[:], dis[:], gate[:], A.mult)

        # ---------- output ----------
        outt = small.tile([1, 2], F32, tag="outt")
        nc.vector.tensor_copy(outt[0:1, 0:1], agg[:])
        nc.vector.tensor_copy(outt[0:1, 1:2], dis[:])
        nc.sync.dma_start(out_ap, outt[:])


def build_nc(num_devices=8):
    nc = bacc.Bacc("TRN2", target_bir_lowering=False, debug=False,
                   num_devices=num_devices)
    sim = nc.dram_tensor("sim", (4, P), F32, kind="ExternalInput")
    tgt = nc.dram_tensor("tgt", (2, P), I32, kind="ExternalInput")
    ne = nc.dram_tensor("ne", (1, NM * NM), F32, kind="ExternalInput")
    miotas = nc.dram_tensor("miotas", (PARTS, 4), F32, kind="ExternalInput")
    bmask = nc.dram_tensor("bmask", (PARTS, PARTS), F32,
                           kind="ExternalInput")
    out = nc.dram_tensor("out", (1, 2), F32, kind="ExternalOutput")
    with tile.TileContext(nc) as tc:
        build_kernel_body(tc, out.ap(), sim.ap(), tgt.ap(), ne.ap(), miotas.ap(), bmask.ap())
    nc.compile()
    return nc


_NC_CACHE = {}


def _ne_const():
    return (1.0 - np.eye(NM, dtype=np.float32)).reshape(1, NM * NM)


def _miotas_const():
    return (np.arange(PARTS)[:, None] % 4 +
            4 * np.arange(4)[None, :]).astype(np.float32)


def _bmask_const():
    bm = np.zeros((PARTS, PARTS), np.float32)
    for g in range(32):
        bm[4 * g:4 * (g + 1), 4 * g:4 * (g + 1)] = 1.0
    return bm


def _get_exec(n_cores):
    """Build the Bass program and a cached jit-compiled SPMD executable."""
    if "fn" in _NC_CACHE:
        return _NC_CACHE
    import jax
    from jax.experimental.shard_map import shard_map
    from jax.sharding import Mesh, PartitionSpec
    from concourse import bass2jax

    bass2jax.install_neuronx_cc_hook()
    nc = build_nc(num_devices=n_cores)

    in_names = []
    out_names = []
    out_avals = []
    zero_outs = []
    for alloc in nc.m.functions[0].allocations:
        if not isinstance(alloc, mybir.MemoryLocationSet):
            continue
        name = alloc.memorylocations[0].name
        if alloc.kind == "ExternalInput":
            if nc.partition_id_tensor is not None and \
                    name == nc.partition_id_tensor.name:
                continue
            in_names.append(name)
        elif alloc.kind == "ExternalOutput":
            shape = tuple(alloc.tensor_shape)
            dtype = mybir.dt.np(alloc.dtype)
            out_names.append(name)
            out_avals.append(jax.core.ShapedArray(shape, dtype))
            zero_outs.append(np.zeros(shape, dtype))
    n_params = len(in_names)
    all_in_names = in_names + out_names
    partition_name = (nc.partition_id_tensor.name
                      if nc.partition_id_tensor is not None else None)
    if partition_name is not None:
        all_in_names = all_in_names + [partition_name]

    def _body(*args):
        operands = list(args)
        if partition_name is not None:
            operands.append(bass2jax.partition_id_tensor())
        outs = bass2jax._bass_exec_p.bind(
            *operands,
            out_avals=tuple(out_avals),
            in_names=tuple(all_in_names),
            out_names=tuple(out_names),
            lowering_input_output_aliases=(),
            sim_require_finite=True,
            sim_require_nnan=True,
            nc=nc,
        )
        return tuple(outs)

    devices = jax.devices()[:n_cores]
    mesh = Mesh(np.asarray(devices), ("core",))
    n_outs = len(out_names)
    fn = jax.jit(
        shard_map(
            _body, mesh=mesh,
            in_specs=(PartitionSpec("core"),) * (n_params + n_outs),
            out_specs=(PartitionSpec("core"),) * n_outs,
            check_rep=False,
        ),
        donate_argnums=tuple(range(n_params, n_params + n_outs)),
        keep_unused=True,
    )
    _NC_CACHE.update(dict(nc=nc, fn=fn, in_names=in_names,
                          out_names=out_names, zero_outs=zero_outs,
                          n_cores=n_cores))
    return _NC_CACHE


def prepare_inputs(preds, targets, n):
    """Concatenated per-core global inputs keyed by dram-parameter name."""
    sim = np.ascontiguousarray(
        preds[:, 2:6].reshape(n * 4, P).astype(np.float32, copy=False))
    tgt = np.ascontiguousarray(
        targets.reshape(n * 2, P).astype(np.int32, copy=False))
    ne = np.tile(_ne_const(), (n, 1))
    miotas = np.tile(_miotas_const(), (n, 1))
    bmask = np.tile(_bmask_const(), (n, 1))
    return {"sim": sim, "tgt": tgt, "ne": ne, "miotas": miotas,
            "bmask": bmask}


def run_prepared(exe, global_ins):
    args = [global_ins[k] for k in exe["in_names"]]
    zeros = [np.zeros((exe["n_cores"] * z.shape[0], *z.shape[1:]), z.dtype)
             for z in exe["zero_outs"]]
    out_arrs = exe["fn"](*args, *zeros)
    return [np.asarray(o) for o in out_arrs]


def kernel(preds: np.ndarray, targets: np.ndarray):
    n = preds.shape[0]
    assert preds.shape == (n, 6, H, W) and targets.shape == (n, 2, H, W)
    exe = _get_exec(n)
    outs = run_prepared(exe, prepare_inputs(preds, targets, n))
    out = outs[exe["out_names"].index("out")].reshape(n, 2)
    return out[:, 0].copy(), out[:, 1].copy()
